# revision 26
# baseline (speedup 1.0000x reference)
"""Trainium2 Bass kernel for nn_BatchPitNorm1d (pairwise Gaussian-CDF KDE + inverse-normal).

Math:  u[b,f] = mean_s Phi((x[b,f] - c[s,f]) / bw[f]),  out = ndtri(u),
       bw = sigmoid(bw_param).

Algorithm (v2): for fixed f, ndtri(u) is a smooth function H_f(x) of x alone,
so instead of B*S*F pairwise Phi evals the kernel:
  1. evaluates the erf-sums g_f(t) at N=12 Chebyshev nodes on a runtime-tight
     domain [-XD, XD] (XD = max|x|), sharded (4 node-groups) x (2 sample
     halves) over 8 cores - NLOC=3 nodes x 1024 samples per core, one fused
     ACT erf instruction per node (accum_out = free-dim sum, per-partition
     scale/bias precomputed on host),
  2. AllGathers the raw [F, NLOC] blocks (feature-major), reads them back
     with one 4D-AP DMA as [F, 2N] and adds the two sample-halves,
  3. applies ndtri at the nodes in feature-major [F, N] layout: central
     rational(3,3) on DVE, deg-5 log-domain tail polynomial on GpSimd (Pool),
     Ln on ACT (table load hidden under the gather), branchless blend via
     copy_predicated,
  4. transposes H via PE, fits per-feature even/odd Chebyshev coefficients
     with one PE matmul,
  5. evaluates y = sum_k ae_k T_k(w) + x~ * sum_k ao_k T_k(w), w = 2x~^2-1,
     with basis tiles T_k / x~T_k precomputed during the grid phase and two
     parallel per-partition-scalar accumulation chains (DVE even, Pool odd).

Host-side prep (cheap [F]-sized math): transpose/shard, x~ = x/XD, w, bw ->
erf scale/bias vectors.  Total error vs the f32 reference: rel ~9e-4
(gate 2e-2).
"""

import math
from contextlib import ExitStack

import numpy as np

import concourse.bass as bass
import concourse.bacc as bacc
import concourse.tile as tile
from concourse import mybir
from concourse import bass_utils

F32 = mybir.dt.float32
ADD = mybir.AluOpType.add
MUL = mybir.AluOpType.mult
SUB = mybir.AluOpType.subtract

N_CORES = 8
B, S, F = 512, 2048, 128
BL = B // N_CORES          # 64 batch rows per core
N_CHEB = 12                # Chebyshev nodes / polynomial order
NGRP = 4                   # node groups (cores 2g, 2g+1 share a node group)
NSPL = 2                   # sample splits (even core: half 0, odd: half 1)
NLOC = N_CHEB // NGRP      # 3 nodes per core
SL = S // NSPL             # 1024 samples per core
J = N_CHEB // 2            # even/odd coefficient count

GSCALE = 1.0 / (2.0 * S)
PLOW = 0.02425             # central/tail blend point (on v = min(u,1-u))
VCLAMP = 0.5 - 2.5e-6      # |q| clamp => v >= 2.5e-6 (empirical node min 5e-6)

# Central branch: ndtri(0.5+q) = q*N(r)/D(r), r = q^2, rational (3,3)
# fitted offline for v >= PLOW (max rel err ~1e-5).
CEN_NUM = [-14.41153095969586, 34.82754843726583, -17.684192118918105,
           2.5066372796948575]
CEN_DEN = [-14.220558591278943, 20.063397583232298, -8.101751140071201, 1.0]

# Tail branch: ndtri(v) = P(ln v), fitted directly in the log domain on
# v in [1.5e-6, 0.0295] (deg 4, max abs err 1.5e-3; node-error sensitivity of
# the final interpolant is ~0.03 rel per unit, so this contributes ~5e-5).
# Coefficients high -> low for the (acc+c)*L Horner form.
TAIL_HL = [6.207629166464076e-05, 0.0028162632922953844, 0.05341104890524519,
           0.7132509118183031, 0.07393079449559067]


def _cheb_theta():
    return (np.arange(N_CHEB) + 0.5) * np.pi / N_CHEB


def _fit_matrix():
    """Cfit[n, k] with alpha[f, k] = sum_n H[f, n] * Cfit[n, k].

    Basis columns 0..J-1 = even coeffs (T_j(w)), J..N-1 = odd (xt*T_j(w)),
    w = 2*xt^2-1, xt = normalized nodes.  XD-independent.
    """
    th = _cheb_theta()
    xt = np.cos(th)
    w = 2 * xt * xt - 1
    M = np.zeros((N_CHEB, N_CHEB))
    for j in range(J):
        M[:, j] = np.cos(j * np.arccos(np.clip(w, -1, 1)))
        M[:, J + j] = xt * M[:, j]
    return np.ascontiguousarray(np.linalg.inv(M).T).astype(np.float32)


def build(with_collective=True, debug_taps=False):
    nc = bacc.Bacc("TRN2", target_bir_lowering=False, debug=False,
                   enable_asserts=False, num_devices=N_CORES)

    # Inputs arrive pre-transposed (feature-major) from the host shard step.
    xw = nc.dram_tensor("xw", [F, 2 * BL], F32, kind="ExternalInput")       # xt | wt
    cdf_t = nc.dram_tensor("cdf_t", [F, SL], F32, kind="ExternalInput")
    consts = nc.dram_tensor("consts", [F, 1 + NLOC], F32, kind="ExternalInput")  # -a | a*t_j
    out = nc.dram_tensor("out", [F, BL], F32, kind="ExternalOutput")
    taps = {}
    if debug_taps:
        for nm, shp in [("d_gacc", [F, NLOC]), ("d_gsum", [F, N_CHEB]),
                        ("d_h", [F, N_CHEB]), ("d_alpha", [F, N_CHEB]),
                        ("d_acce", [F, BL]), ("d_acco", [F, BL])]:
            taps[nm] = nc.dram_tensor(nm, shp, F32, kind="ExternalOutput")

    cfit_h = nc.inline_tensor(_fit_matrix(), name="cfit")
    ident_h = nc.inline_tensor(np.eye(F, dtype=np.float32), name="ident")

    with tile.TileContext(nc) as tc, ExitStack() as ctx:
        sb = ctx.enter_context(tc.tile_pool(name="sb", bufs=1))
        psum = ctx.enter_context(tc.tile_pool(name="psum", bufs=1, space="PSUM"))
        dram = ctx.enter_context(tc.tile_pool(name="dram", bufs=1, space="DRAM"))

        D = nc.vector    # DVE
        P = nc.gpsimd    # Pool
        A = nc.scalar    # ACT
        SP = nc.sync     # SP

        def ts(eng, name, in0, s1, s2=None, op0=MUL, op1=ADD, w=N_CHEB):
            t = sb.tile([F, w], F32, name=name, tag=name)
            if s2 is None:
                eng.tensor_scalar(out=t, in0=in0, scalar1=s1, scalar2=None, op0=op0)
            else:
                eng.tensor_scalar(out=t, in0=in0, scalar1=s1, scalar2=s2,
                                  op0=op0, op1=op1)
            return t

        def stt(eng, name, in0, s, in1, op0=ADD, op1=MUL, w=N_CHEB, out=None):
            t = out if out is not None else sb.tile([F, w], F32, name=name, tag=name)
            eng.scalar_tensor_tensor(out=t, in0=in0, scalar=s, in1=in1,
                                     op0=op0, op1=op1)
            return t

        # ---------------- input DMAs (one per queue, issued up front)
        # cdf in two chunks so the first erf round starts ~0.75us earlier,
        # overlapping the second chunk's transfer with erf compute.
        SH = SL // 2
        cT = sb.tile([F, SL], F32, name="cT")
        SP.dma_start(out=cT[:, :SH], in_=cdf_t[:, :SH])
        SP.dma_start(out=cT[:, SH:], in_=cdf_t[:, SH:])
        cst = sb.tile([F, 1 + NLOC], F32, name="cst")
        P.dma_start(out=cst, in_=consts[:, :])
        xw_sb = sb.tile([F, 2 * BL], F32, name="xw")
        P.dma_start(out=xw_sb, in_=xw[:, :])
        cfit_sb = sb.tile([N_CHEB, N_CHEB], F32, name="cfit")
        P.dma_start(out=cfit_sb, in_=cfit_h[:, :])
        ident_sb = sb.tile([F, F], F32, name="ident")
        P.dma_start(out=ident_sb, in_=ident_h[:, :])

        xt = xw_sb[:, :BL]
        wt = xw_sb[:, BL:]

        half_c = sb.tile([F, 1], F32, name="halfc")
        D.memset(half_c, 0.5)
        # Data-independent dummy erf: forces the erf table load at t~1us,
        # while the cdf DMA is still in flight (instead of right before erf0).
        erfdum = sb.tile([F, 1], F32, name="erfdum")
        A.activation(out=erfdum, in_=half_c,
                     func=mybir.ActivationFunctionType.Erf, scale=0.0,
                     bias=half_c[:, 0:1])

        # ---------------- basis precompute (hidden under grid phase)
        # Even basis T_k(w), odd basis xt*T_k(w); T0=1 and xT0=xt are implicit.
        wt2 = ts(D, "wt2", wt, 2.0, w=BL)
        wsq = stt(D, "wsq", wt, 0.0, wt, w=BL)
        T2 = ts(D, "T2", wsq, 2.0, -1.0, w=BL)
        Tk = {1: wt, 2: T2}
        for k in range(3, J):
            p = stt(D, f"Tp{k}", Tk[k - 1], 0.0, wt2, w=BL)
            Tk[k] = stt(D, f"T{k}", p, 0.0, Tk[k - 2], op1=SUB, w=BL)
        xTk = {}
        for k in range(1, J):
            xTk[k] = stt(D, f"xT{k}", Tk[k], 0.0, xt, w=BL)

        # ---------------- grid: gacc[f, j] = sum_s erf(-a_f*c_sf + a_f*t_j)
        # Two rounds (one per cdf chunk), partial sums merged with one STT.
        gacc2 = sb.tile([F, 2 * NLOC], F32, name="gacc2")
        scr = psum.tile([F, SH], F32, name="scr", tag="scr")
        for rnd in range(2):
            for j in range(NLOC):
                A.activation(out=scr, in_=cT[:, rnd * SH:(rnd + 1) * SH],
                             func=mybir.ActivationFunctionType.Erf,
                             bias=cst[:, 1 + j:2 + j], scale=cst[:, 0:1],
                             accum_out=gacc2[:, rnd * NLOC + j:rnd * NLOC + j + 1])
        gacc = stt(D, "gacc", gacc2[:, :NLOC], 0.0, gacc2[:, NLOC:], op1=ADD,
                   w=NLOC)
        # Force the Ln table switch right after the grid so the ~1.3us load
        # hides under the gather round-trip.  Reads the last accum column so
        # the scheduler cannot hoist it between the erfs (which would force
        # extra erf-table reloads).
        lndum = sb.tile([F, 1], F32, name="lndum")
        A.activation(out=lndum, in_=gacc2[:, 2 * NLOC - 1:2 * NLOC],
                     func=mybir.ActivationFunctionType.Ln, scale=0.0,
                     bias=half_c[:, 0:1])

        # ---------------- exchange: AllGather of the [F, NLOC] blocks
        cin = dram.tile([F, NLOC], F32, tag="cin")
        SP.dma_start(out=cin[:, :], in_=gacc)
        cout = dram.tile([N_CORES, F, NLOC], F32, tag="cout",
                         addr_space="Shared" if with_collective else "Local")
        if with_collective:
            P.collective_compute(
                "AllGather", mybir.AluOpType.bypass,
                replica_groups=[list(range(N_CORES))],
                ins=[cin.opt()], outs=[cout.opt()],
            )
        # Single readback of all 8 [F, NLOC] blocks, rank-major:
        # gbig[f, rank*NLOC + j] = cout[rank][f][j], rank = g*NSPL + h.
        gbig = sb.tile([F, N_CORES * NLOC], F32, name="gbig")
        if with_collective:
            src_ap = bass.AP(
                tensor=cout.tensor, offset=cout.offset,
                ap=[[NLOC, F], [F * NLOC, N_CORES], [1, NLOC]])
        else:  # stand-in: broadcast-read own block (timing model only)
            src_ap = bass.AP(
                tensor=cin.tensor, offset=cin.offset,
                ap=[[NLOC, F], [0, N_CORES], [1, NLOC]])
        A.dma_start(out=gbig[:, :], in_=src_ap)

        # g_sum[f, g*NLOC+j] = sum_h gbig[f, (g*NSPL+h)*NLOC + j]
        g_sum = sb.tile([F, N_CHEB], F32, name="gsum")
        gb_w = N_CORES * NLOC
        h0_ap = bass.AP(tensor=gbig.tensor, offset=gbig.offset,
                        ap=[[gb_w, F], [NSPL * NLOC, NGRP], [1, NLOC]])
        h1_ap = bass.AP(tensor=gbig.tensor, offset=gbig.offset + NLOC,
                        ap=[[gb_w, F], [NSPL * NLOC, NGRP], [1, NLOC]])
        D.scalar_tensor_tensor(out=g_sum, in0=h0_ap, scalar=0.0, in1=h1_ap,
                               op0=ADD, op1=ADD)

        # ---------------- ndtri at the nodes, feature-major [F, N]
        # gscale = 1/(2S) = 2^-12 is an exact power of two, so it is folded
        # into the rational coefficients (exact f32 scaling): work directly on
        # r' = g^2 and finish with *g instead of computing q = g*gscale.
        CN = [CEN_NUM[i] * GSCALE ** (2 * (3 - i) + 1) for i in range(4)]
        CD = [CEN_DEN[i] * GSCALE ** (2 * (3 - i)) for i in range(4)]
        r2 = stt(D, "r2", g_sum, 0.0, g_sum)
        # |q| = |g|*gscale on ACT (Abs is in every table set); v = 0.5 - |q|
        # stays >= ~5e-6 for this data (empirical node minimum).
        mn2 = sb.tile([F, N_CHEB], F32, name="mn2")
        A.activation(out=mn2, in_=g_sum, func=mybir.ActivationFunctionType.Abs,
                     scale=GSCALE)
        mc = sb.tile([F, N_CHEB], mybir.dt.uint8, name="mc")
        D.tensor_scalar(out=mc, in0=mn2, scalar1=0.5 - PLOW, scalar2=None,
                        op0=mybir.AluOpType.is_le)
        # ACT: lnv = Ln(0.5 - |q|)
        lnv = sb.tile([F, N_CHEB], F32, name="lnv")
        A.activation(out=lnv, in_=mn2, func=mybir.ActivationFunctionType.Ln,
                     scale=-1.0, bias=half_c[:, 0:1])
        # central: q*N(r)/D(r) in the scaled variables
        ca = ts(D, "ca0", r2, float(CN[0]))
        ca = stt(D, "ca1", ca, float(CN[1]), r2)
        ca = stt(D, "ca2", ca, float(CN[2]), r2)
        nq = stt(D, "nq", ca, float(CN[3]), g_sum)
        da = ts(D, "da0", r2, float(CD[0]))
        da = stt(D, "da1", da, float(CD[1]), r2)
        da = stt(D, "da2", da, float(CD[2]), r2)
        df = ts(D, "df", da, float(CD[3]), None, op0=ADD)
        rec = sb.tile([F, N_CHEB], F32, name="rec")
        D.reciprocal(out=rec, in_=df)
        xc = stt(D, "xc", nq, 0.0, rec)
        # tail: P(ln v) * (-sign(g)); Sign is in every ACT table set
        nsgn = sb.tile([F, N_CHEB], F32, name="nsgn")
        A.activation(out=nsgn, in_=g_sum,
                     func=mybir.ActivationFunctionType.Sign, scale=-1.0)
        ta = ts(D, "ta0", lnv, float(TAIL_HL[0]))
        for i, c in enumerate(TAIL_HL[1:-1]):
            ta = stt(D, f"ta{i + 1}", ta, float(c), lnv)
        h = sb.tile([F, N_CHEB], F32, name="h")
        stt(D, "tsgn", ta, float(TAIL_HL[-1]), nsgn, out=h)
        # blend: overwrite central region with xc
        D.copy_predicated(h, mc, xc)

        # ---------------- fit: alpha = h @ Cfit via PE transpose + matmul
        hT_ps = psum.tile([N_CHEB, F], F32, tag="hT")
        nc.tensor.transpose(hT_ps, h, ident_sb)
        hT_sb = sb.tile([N_CHEB, F], F32, name="hT")
        A.activation(out=hT_sb, in_=hT_ps,
                     func=mybir.ActivationFunctionType.Copy)
        alpha_ps = psum.tile([F, N_CHEB], F32, tag="alpha")
        nc.tensor.matmul(out=alpha_ps, lhsT=hT_sb, rhs=cfit_sb,
                         start=True, stop=True)
        alpha = sb.tile([F, N_CHEB], F32, name="alpha")
        D.tensor_copy(out=alpha, in_=alpha_ps)

        # ---------------- evaluate: y = sum_k ae_k T_k(w) + sum_k ao_k xt T_k(w)
        # All terms as DVE multiply-accumulate chains; several independent
        # chains interleave on the engine so the per-op write-ack pipelines.
        # Term list: (basis tile, alpha column); ae0 (constant) fused at the end.
        terms = [(wt, 1)] + [(Tk[k], k) for k in range(2, J)]          # even
        terms += [(xt, J)] + [(xTk[k], J + k) for k in range(1, J)]    # odd
        NCH = 3
        chains = []
        for c in range(NCH):
            sub = terms[c::NCH]
            acc = ts(D, f"acc{c}0", sub[0][0], alpha[:, sub[0][1]:sub[0][1] + 1],
                     None, w=BL)
            for i, (bt, col) in enumerate(sub[1:]):
                acc = stt(D, f"acc{c}{i + 1}", bt, alpha[:, col:col + 1], acc,
                          op0=MUL, op1=ADD, w=BL)
            chains.append(acc)
        y01 = stt(D, "y01", chains[0], alpha[:, 0:1], chains[1], op0=ADD,
                  op1=ADD, w=BL)
        y = stt(D, "y", y01, 0.0, chains[2], op0=ADD, op1=ADD, w=BL)

        SP.dma_start(out=out[:, :], in_=y)

        if debug_taps:
            for nm, t in [("d_gacc", gacc), ("d_gsum", g_sum), ("d_h", h),
                          ("d_alpha", alpha), ("d_acce", ye),
                          ("d_acco", acc_o)]:
                SP.dma_start(out=taps[nm][:, :], in_=t)

    nc.compile()
    return nc


_CACHE = {}


def _get_nc():
    if "nc" not in _CACHE:
        _CACHE["nc"] = build(with_collective=True)
    return _CACHE["nc"]


def kernel(x, cdf_data, bw_param):
    x = np.ascontiguousarray(x, dtype=np.float32)
    cdf_data = np.ascontiguousarray(cdf_data, dtype=np.float32)
    bw_param = np.ascontiguousarray(bw_param, dtype=np.float32)
    nc = _get_nc()

    xd = float(np.abs(x).max()) * 1.0005
    th = _cheb_theta()
    t_nodes = (xd * np.cos(th)).astype(np.float32)              # [N]
    bw = (1.0 / (1.0 + np.exp(-bw_param.astype(np.float64))))[0]
    a = (1.0 / (bw * math.sqrt(2.0))).astype(np.float32)        # [F]

    xt = np.clip(x.T, -xd, xd).astype(np.float32) / np.float32(xd)   # [F, B]
    wtf = (np.float32(2.0) * xt * xt - np.float32(1.0)).astype(np.float32)
    cdf_halves = [np.ascontiguousarray(cdf_data[h * SL:(h + 1) * SL].T)
                  for h in range(NSPL)]                          # each [F, SL]

    in_maps = []
    for i in range(N_CORES):
        g, h = i // NSPL, i % NSPL
        xw_i = np.concatenate([xt[:, i * BL:(i + 1) * BL],
                               wtf[:, i * BL:(i + 1) * BL]], axis=1)
        bias = a[:, None] * t_nodes[None, g * NLOC:(g + 1) * NLOC]  # [F, NLOC]
        consts_i = np.concatenate([-a[:, None], bias], axis=1)
        in_maps.append({
            "xw": np.ascontiguousarray(xw_i),
            "cdf_t": cdf_halves[h],
            "consts": np.ascontiguousarray(consts_i.astype(np.float32)),
        })
    res = bass_utils.run_bass_kernel_spmd(nc, in_maps, core_ids=list(range(N_CORES)))
    return np.concatenate([res.results[i]["out"].T for i in range(N_CORES)], axis=0)


# revision 34
# speedup vs baseline: 1.0373x; 1.0373x over previous
"""Trainium2 Bass kernel for nn_BatchPitNorm1d (pairwise Gaussian-CDF KDE + inverse-normal).

Math:  u[b,f] = mean_s Phi((x[b,f] - c[s,f]) / bw[f]),  out = ndtri(u),
       bw = sigmoid(bw_param).

Algorithm (v2): for fixed f, ndtri(u) is a smooth function H_f(x) of x alone,
so instead of B*S*F pairwise Phi evals the kernel:
  1. evaluates the erf-sums g_f(t) at N=12 Chebyshev nodes on a runtime-tight
     domain [-XD, XD] (XD = max|x|), sharded (4 node-groups) x (2 sample
     halves) over 8 cores - NLOC=3 nodes x 1024 samples per core, one fused
     ACT erf instruction per node (accum_out = free-dim sum, per-partition
     scale/bias precomputed on host),
  2. AllGathers the raw [F, NLOC] blocks (feature-major), reads them back
     with one 4D-AP DMA as [F, 2N] and adds the two sample-halves,
  3. applies ndtri at the nodes in feature-major [F, N] layout: central
     rational(3,3) on DVE, deg-5 log-domain tail polynomial on GpSimd (Pool),
     Ln on ACT (table load hidden under the gather), branchless blend via
     copy_predicated,
  4. transposes H via PE, fits per-feature even/odd Chebyshev coefficients
     with one PE matmul,
  5. evaluates y = sum_k ae_k T_k(w) + x~ * sum_k ao_k T_k(w), w = 2x~^2-1,
     with basis tiles T_k / x~T_k precomputed during the grid phase and two
     parallel per-partition-scalar accumulation chains (DVE even, Pool odd).

Host-side prep (cheap [F]-sized math): transpose/shard, x~ = x/XD, w, bw ->
erf scale/bias vectors.  Total error vs the f32 reference: rel ~9e-4
(gate 2e-2).
"""

import math
from contextlib import ExitStack

import numpy as np

import concourse.bass as bass
import concourse.bacc as bacc
import concourse.tile as tile
from concourse import mybir
from concourse import bass_utils

F32 = mybir.dt.float32
ADD = mybir.AluOpType.add
MUL = mybir.AluOpType.mult
SUB = mybir.AluOpType.subtract

N_CORES = 8
B, S, F = 512, 2048, 128
BL = B // N_CORES          # 64 batch rows per core
N_CHEB = 12                # Chebyshev nodes / polynomial order
NGRP = 4                   # node groups (cores 2g, 2g+1 share a node group)
NSPL = 2                   # sample splits (even core: half 0, odd: half 1)
NLOC = N_CHEB // NGRP      # 3 nodes per core
SL = S // NSPL             # 1024 samples per core
J = N_CHEB // 2            # even/odd coefficient count

GSCALE = 1.0 / (2.0 * S)
PLOW = 0.02425             # central/tail blend point (on v = min(u,1-u))
VCLAMP = 0.5 - 2.5e-6      # |q| clamp => v >= 2.5e-6 (empirical node min 5e-6)

# Central branch: ndtri(0.5+q) = q*N(r)/D(r), r = q^2, rational (3,2)
# fitted offline for v >= PLOW (max rel err 1.3e-3 -> ~1e-4 in the final
# interpolant, far below the N=12 truncation error).  Coeffs high -> low.
CEN_NUM = [3.230685621370267, 10.518986770753806, -12.088481781706497,
           2.5066266687310588]
CEN_DEN = [8.04930843954961, -5.869995381222777, 1.0]

# Tail branch: ndtri(v) = P(ln v), fitted directly in the log domain on
# v in [1.5e-6, 0.0295] (deg 4, max abs err 1.5e-3; node-error sensitivity of
# the final interpolant is ~0.03 rel per unit, so this contributes ~5e-5).
# Coefficients high -> low for the (acc+c)*L Horner form.
TAIL_HL = [6.207629166464076e-05, 0.0028162632922953844, 0.05341104890524519,
           0.7132509118183031, 0.07393079449559067]


def _cheb_theta():
    return (np.arange(N_CHEB) + 0.5) * np.pi / N_CHEB


def _fit_matrix():
    """Cfit[n, k] with alpha[f, k] = sum_n H[f, n] * Cfit[n, k].

    Basis columns 0..J-1 = even coeffs (T_j(w)), J..N-1 = odd (xt*T_j(w)),
    w = 2*xt^2-1, xt = normalized nodes.  XD-independent.
    """
    th = _cheb_theta()
    xt = np.cos(th)
    w = 2 * xt * xt - 1
    M = np.zeros((N_CHEB, N_CHEB))
    for j in range(J):
        M[:, j] = np.cos(j * np.arccos(np.clip(w, -1, 1)))
        M[:, J + j] = xt * M[:, j]
    return np.ascontiguousarray(np.linalg.inv(M).T).astype(np.float32)


def build(with_collective=True, debug_taps=False):
    nc = bacc.Bacc("TRN2", target_bir_lowering=False, debug=False,
                   enable_asserts=False, num_devices=N_CORES)

    # Inputs arrive pre-transposed (feature-major) from the host shard step.
    xw = nc.dram_tensor("xw", [F, 2 * BL], F32, kind="ExternalInput")       # xt | wt
    cdf_t = nc.dram_tensor("cdf_t", [F, SL], F32, kind="ExternalInput")
    consts = nc.dram_tensor("consts", [F, 1 + NLOC], F32, kind="ExternalInput")  # -a | a*t_j
    out = nc.dram_tensor("out", [F, BL], F32, kind="ExternalOutput")
    taps = {}
    if debug_taps:
        for nm, shp in [("d_gacc", [F, NLOC]), ("d_gsum", [F, N_CHEB]),
                        ("d_h", [F, N_CHEB]), ("d_alpha", [F, N_CHEB]),
                        ("d_acce", [F, BL]), ("d_acco", [F, BL])]:
            taps[nm] = nc.dram_tensor(nm, shp, F32, kind="ExternalOutput")

    cfit_h = nc.inline_tensor(_fit_matrix(), name="cfit")
    ident_h = nc.inline_tensor(np.eye(F, dtype=np.float32), name="ident")

    with tile.TileContext(nc) as tc, ExitStack() as ctx:
        sb = ctx.enter_context(tc.tile_pool(name="sb", bufs=1))
        psum = ctx.enter_context(tc.tile_pool(name="psum", bufs=1, space="PSUM"))
        dram = ctx.enter_context(tc.tile_pool(name="dram", bufs=1, space="DRAM"))

        D = nc.vector    # DVE
        P = nc.gpsimd    # Pool
        A = nc.scalar    # ACT
        SP = nc.sync     # SP

        def ts(eng, name, in0, s1, s2=None, op0=MUL, op1=ADD, w=N_CHEB):
            t = sb.tile([F, w], F32, name=name, tag=name)
            if s2 is None:
                eng.tensor_scalar(out=t, in0=in0, scalar1=s1, scalar2=None, op0=op0)
            else:
                eng.tensor_scalar(out=t, in0=in0, scalar1=s1, scalar2=s2,
                                  op0=op0, op1=op1)
            return t

        def stt(eng, name, in0, s, in1, op0=ADD, op1=MUL, w=N_CHEB, out=None):
            t = out if out is not None else sb.tile([F, w], F32, name=name, tag=name)
            eng.scalar_tensor_tensor(out=t, in0=in0, scalar=s, in1=in1,
                                     op0=op0, op1=op1)
            return t

        # ---------------- input DMAs (one per queue, issued up front)
        cT = sb.tile([F, SL], F32, name="cT")
        SP.dma_start(out=cT, in_=cdf_t[:, :])
        cst = sb.tile([F, 1 + NLOC], F32, name="cst")
        P.dma_start(out=cst, in_=consts[:, :])
        xw_sb = sb.tile([F, 2 * BL], F32, name="xw")
        P.dma_start(out=xw_sb, in_=xw[:, :])
        cfit_sb = sb.tile([N_CHEB, N_CHEB], F32, name="cfit")
        P.dma_start(out=cfit_sb, in_=cfit_h[:, :])
        ident_sb = sb.tile([F, F], F32, name="ident")
        P.dma_start(out=ident_sb, in_=ident_h[:, :])

        xt = xw_sb[:, :BL]
        wt = xw_sb[:, BL:]

        half_c = sb.tile([F, 1], F32, name="halfc")
        D.memset(half_c, 0.5)
        # Data-independent dummy erf: forces the erf table load at t~1us,
        # while the cdf DMA is still in flight (instead of right before erf0).
        erfdum = sb.tile([F, 1], F32, name="erfdum")
        A.activation(out=erfdum, in_=half_c,
                     func=mybir.ActivationFunctionType.Erf, scale=0.0,
                     bias=half_c[:, 0:1])

        # ---------------- basis precompute (hidden under grid phase)
        # Even basis T_k(w), odd basis xt*T_k(w); T0=1 and xT0=xt are implicit.
        wt2 = ts(D, "wt2", wt, 2.0, w=BL)
        wsq = stt(D, "wsq", wt, 0.0, wt, w=BL)
        T2 = ts(D, "T2", wsq, 2.0, -1.0, w=BL)
        Tk = {1: wt, 2: T2}
        for k in range(3, J):
            p = stt(D, f"Tp{k}", Tk[k - 1], 0.0, wt2, w=BL)
            Tk[k] = stt(D, f"T{k}", p, 0.0, Tk[k - 2], op1=SUB, w=BL)
        xTk = {}
        for k in range(1, J):
            xTk[k] = stt(D, f"xT{k}", Tk[k], 0.0, xt, w=BL)

        # ---------------- grid: gacc[f, j] = sum_s erf(-a_f*c_sf + a_f*t_j)
        gacc = sb.tile([F, NLOC], F32, name="gacc")
        scr = psum.tile([F, SL], F32, name="scr", tag="scr")
        for j in range(NLOC):
            A.activation(out=scr, in_=cT, func=mybir.ActivationFunctionType.Erf,
                         bias=cst[:, 1 + j:2 + j], scale=cst[:, 0:1],
                         accum_out=gacc[:, j:j + 1])
        # Force the Ln table switch right after the grid so the ~1.3us load
        # hides under the gather round-trip.  Reads the last accum column so
        # the scheduler cannot hoist it between the erfs (which would force
        # extra erf-table reloads).
        lndum = sb.tile([F, 1], F32, name="lndum")
        A.activation(out=lndum, in_=gacc[:, NLOC - 1:NLOC],
                     func=mybir.ActivationFunctionType.Ln, scale=0.0,
                     bias=half_c[:, 0:1])

        # ---------------- exchange: AllGather of the [F, NLOC] blocks
        cin = dram.tile([F, NLOC], F32, tag="cin")
        SP.dma_start(out=cin[:, :], in_=gacc)
        cout = dram.tile([N_CORES, F, NLOC], F32, tag="cout",
                         addr_space="Shared" if with_collective else "Local")
        if with_collective:
            P.collective_compute(
                "AllGather", mybir.AluOpType.bypass,
                replica_groups=[list(range(N_CORES))],
                ins=[cin.opt()], outs=[cout.opt()],
            )
        # Single readback of all 8 [F, NLOC] blocks, rank-major:
        # gbig[f, rank*NLOC + j] = cout[rank][f][j], rank = g*NSPL + h.
        gbig = sb.tile([F, N_CORES * NLOC], F32, name="gbig")
        if with_collective:
            src_ap = bass.AP(
                tensor=cout.tensor, offset=cout.offset,
                ap=[[NLOC, F], [F * NLOC, N_CORES], [1, NLOC]])
        else:  # stand-in: broadcast-read own block (timing model only)
            src_ap = bass.AP(
                tensor=cin.tensor, offset=cin.offset,
                ap=[[NLOC, F], [0, N_CORES], [1, NLOC]])
        SP.dma_start(out=gbig[:, :], in_=src_ap)

        # g_sum[f, g*NLOC+j] = sum_h gbig[f, (g*NSPL+h)*NLOC + j]
        g_sum = sb.tile([F, N_CHEB], F32, name="gsum")
        gb_w = N_CORES * NLOC
        h0_ap = bass.AP(tensor=gbig.tensor, offset=gbig.offset,
                        ap=[[gb_w, F], [NSPL * NLOC, NGRP], [1, NLOC]])
        h1_ap = bass.AP(tensor=gbig.tensor, offset=gbig.offset + NLOC,
                        ap=[[gb_w, F], [NSPL * NLOC, NGRP], [1, NLOC]])
        D.scalar_tensor_tensor(out=g_sum, in0=h0_ap, scalar=0.0, in1=h1_ap,
                               op0=ADD, op1=ADD)

        # ---------------- ndtri at the nodes, feature-major [F, N]
        # gscale = 1/(2S) = 2^-12 is an exact power of two, so it is folded
        # into the rational coefficients (exact f32 scaling): work directly on
        # r' = g^2 and finish with *g instead of computing q = g*gscale.
        CN = [CEN_NUM[i] * GSCALE ** (2 * (3 - i) + 1) for i in range(4)]
        CD = [CEN_DEN[i] * GSCALE ** (2 * (2 - i)) for i in range(3)]
        r2 = stt(D, "r2", g_sum, 0.0, g_sum)
        # |q| = |g|*gscale on ACT (Abs is in every table set); v = 0.5 - |q|
        # stays >= ~5e-6 for this data (empirical node minimum).
        mn2 = sb.tile([F, N_CHEB], F32, name="mn2")
        A.activation(out=mn2, in_=g_sum, func=mybir.ActivationFunctionType.Abs,
                     scale=GSCALE)
        mc = sb.tile([F, N_CHEB], mybir.dt.uint8, name="mc")
        D.tensor_scalar(out=mc, in0=mn2, scalar1=0.5 - PLOW, scalar2=None,
                        op0=mybir.AluOpType.is_le)
        # ACT: lnv = Ln(0.5 - |q|)
        lnv = sb.tile([F, N_CHEB], F32, name="lnv")
        A.activation(out=lnv, in_=mn2, func=mybir.ActivationFunctionType.Ln,
                     scale=-1.0, bias=half_c[:, 0:1])
        # central: q*N(r)/D(r) in the scaled variables
        ca = ts(D, "ca0", r2, float(CN[0]))
        ca = stt(D, "ca1", ca, float(CN[1]), r2)
        ca = stt(D, "ca2", ca, float(CN[2]), r2)
        nq = stt(D, "nq", ca, float(CN[3]), g_sum)
        da = ts(D, "da0", r2, float(CD[0]))
        da = stt(D, "da1", da, float(CD[1]), r2)
        df = ts(D, "df", da, float(CD[2]), None, op0=ADD)
        rec = sb.tile([F, N_CHEB], F32, name="rec")
        D.reciprocal(out=rec, in_=df)
        xc = stt(D, "xc", nq, 0.0, rec)
        # tail: P(ln v) * (-sign(g)); Sign is in every ACT table set
        nsgn = sb.tile([F, N_CHEB], F32, name="nsgn")
        A.activation(out=nsgn, in_=g_sum,
                     func=mybir.ActivationFunctionType.Sign, scale=-1.0)
        ta = ts(D, "ta0", lnv, float(TAIL_HL[0]))
        for i, c in enumerate(TAIL_HL[1:-1]):
            ta = stt(D, f"ta{i + 1}", ta, float(c), lnv)
        h = sb.tile([F, N_CHEB], F32, name="h")
        stt(D, "tsgn", ta, float(TAIL_HL[-1]), nsgn, out=h)
        # blend: overwrite central region with xc
        D.copy_predicated(h, mc, xc)

        # ---------------- fit: alpha = h @ Cfit via PE transpose + matmul
        hT_ps = psum.tile([N_CHEB, F], F32, tag="hT")
        nc.tensor.transpose(hT_ps, h, ident_sb)
        hT_sb = sb.tile([N_CHEB, F], F32, name="hT")
        A.activation(out=hT_sb, in_=hT_ps,
                     func=mybir.ActivationFunctionType.Copy)
        alpha_ps = psum.tile([F, N_CHEB], F32, tag="alpha")
        nc.tensor.matmul(out=alpha_ps, lhsT=hT_sb, rhs=cfit_sb,
                         start=True, stop=True)
        alpha = sb.tile([F, N_CHEB], F32, name="alpha")
        D.tensor_copy(out=alpha, in_=alpha_ps)

        # ---------------- evaluate: y = sum_k ae_k T_k(w) + sum_k ao_k xt T_k(w)
        # All terms as DVE multiply-accumulate chains; several independent
        # chains interleave on the engine so the per-op write-ack pipelines.
        # Term list: (basis tile, alpha column); ae0 (constant) fused at the end.
        terms = [(wt, 1)] + [(Tk[k], k) for k in range(2, J)]          # even
        terms += [(xt, J)] + [(xTk[k], J + k) for k in range(1, J)]    # odd
        NCH = 3
        chains = []
        for c in range(NCH):
            sub = terms[c::NCH]
            acc = ts(D, f"acc{c}0", sub[0][0], alpha[:, sub[0][1]:sub[0][1] + 1],
                     None, w=BL)
            for i, (bt, col) in enumerate(sub[1:]):
                acc = stt(D, f"acc{c}{i + 1}", bt, alpha[:, col:col + 1], acc,
                          op0=MUL, op1=ADD, w=BL)
            chains.append(acc)
        y01 = stt(D, "y01", chains[0], alpha[:, 0:1], chains[1], op0=ADD,
                  op1=ADD, w=BL)
        y = stt(D, "y", y01, 0.0, chains[2], op0=ADD, op1=ADD, w=BL)

        SP.dma_start(out=out[:, :], in_=y)

        if debug_taps:
            for nm, t in [("d_gacc", gacc), ("d_gsum", g_sum), ("d_h", h),
                          ("d_alpha", alpha), ("d_acce", ye),
                          ("d_acco", acc_o)]:
                SP.dma_start(out=taps[nm][:, :], in_=t)

    nc.compile()
    return nc


_CACHE = {}


def _get_nc():
    if "nc" not in _CACHE:
        _CACHE["nc"] = build(with_collective=True)
    return _CACHE["nc"]


def kernel(x, cdf_data, bw_param):
    x = np.ascontiguousarray(x, dtype=np.float32)
    cdf_data = np.ascontiguousarray(cdf_data, dtype=np.float32)
    bw_param = np.ascontiguousarray(bw_param, dtype=np.float32)
    nc = _get_nc()

    xd = float(np.abs(x).max()) * 1.0005
    th = _cheb_theta()
    t_nodes = (xd * np.cos(th)).astype(np.float32)              # [N]
    bw = (1.0 / (1.0 + np.exp(-bw_param.astype(np.float64))))[0]
    a = (1.0 / (bw * math.sqrt(2.0))).astype(np.float32)        # [F]

    xt = np.clip(x.T, -xd, xd).astype(np.float32) / np.float32(xd)   # [F, B]
    wtf = (np.float32(2.0) * xt * xt - np.float32(1.0)).astype(np.float32)
    cdf_halves = [np.ascontiguousarray(cdf_data[h * SL:(h + 1) * SL].T)
                  for h in range(NSPL)]                          # each [F, SL]

    in_maps = []
    for i in range(N_CORES):
        g, h = i // NSPL, i % NSPL
        xw_i = np.concatenate([xt[:, i * BL:(i + 1) * BL],
                               wtf[:, i * BL:(i + 1) * BL]], axis=1)
        bias = a[:, None] * t_nodes[None, g * NLOC:(g + 1) * NLOC]  # [F, NLOC]
        consts_i = np.concatenate([-a[:, None], bias], axis=1)
        in_maps.append({
            "xw": np.ascontiguousarray(xw_i),
            "cdf_t": cdf_halves[h],
            "consts": np.ascontiguousarray(consts_i.astype(np.float32)),
        })
    res = bass_utils.run_bass_kernel_spmd(nc, in_maps, core_ids=list(range(N_CORES)))
    return np.concatenate([res.results[i]["out"].T for i in range(N_CORES)], axis=0)
